# revision 45
# baseline (speedup 1.0000x reference)
"""Grouped Conv2d (512 groups, 2->2 ch/group, 3x3 VALID) on 8 trn2 NeuronCores.

Strategy (hybrid, fp16 data path):
  - Shard the 512 groups across 8 cores: 64 groups = 128 channels per core.
    Fully independent (no collectives); batch stays whole on every core.
  - Row-split each batch's 54 output rows across engines:
      * PE rows [0, R_PE): block-diagonal 128x128 weight per 3x3 tap; 9
        accumulating fp16 matmuls per PSUM chunk (<=9 rows x 54 cols); ACT
        evicts PSUM -> fp16 SBUF (interleaved with its product ops).
      * DVE rows: per-term tensor_scalar product (4x mode) + tensor_tensor
        accumulate (2x mode); 18 terms = 9 taps x {diag, pair-swapped}.
      * ACT-product rows: ACT per-partition-scale products, gpsimd adds.
    Per-group 2x2 channel mixing needs x[p^1] at partition p: host supplies
    a channel pair-swapped copy `xs` of the bottom x rows.
  - K batches are fused per vector-engine op (4D access patterns) to
    amortize per-op fixed costs; the PE still works chunk-by-chunk per
    batch. The next super-batch's loads are emitted before this one's
    final stores so prefetch is never queued behind them.
  - fp16 halves DMA bytes vs fp32 (x converted on host; y back on host).
"""

import sys

import numpy as np

for _p in ("/opt/trn_rl_repo",):
    if _p not in sys.path:
        sys.path.insert(0, _p)

import concourse.bacc as bacc
import concourse.bass as bass
import concourse.tile as tile
from concourse import mybir
from concourse.bass_utils import run_bass_kernel_spmd

N_CORES = 8
B, C, H, W = 16, 1024, 56, 56
KH = KW = 3
HO, WO = H - KH + 1, W - KW + 1  # 54, 54
CPC = C // N_CORES  # 128 channels (64 groups) per core

# Row split (mutable for sweeps; kernel() uses these values at build time)
CFG = {
    "K": 2,       # batches fused per vector-engine op
    "R_PE": 42,   # rows on the PE (psum chunks of <=9 rows)
    "R_DVE": 9,   # rows accumulated on DVE
    # rest of the 54 rows: produced by ACT, added on Pool
    "POOL_TT": 0,  # DVE-row add-terms offloaded to gpsimd (in-chain; slow)
    "OV2": 0,      # DVE-row terms accumulated out-of-chain (ov2, merged once)
    "WARMUP": 16,  # dummy matmuls before batch 0
}

_NC_CACHE = {}

HSPLIT = 30
OSPLIT = 27


def _chunks(r_pe):
    out, r = [], 0
    while r < r_pe:
        out.append((r, min(9, r_pe - r)))
        r += min(9, r_pe - r)
    return out


def _build_program(cfg):
    k = cfg["K"]
    assert B % k == 0
    r_pe = cfg["R_PE"]
    r_dve = cfg["R_DVE"]
    r_act = HO - r_pe - r_dve
    assert r_dve > 0 and r_act >= 0
    vstart = r_pe
    xs_rows = HO - r_pe + KH - 1
    chunks = _chunks(r_pe)

    nc = bacc.Bacc(
        "TRN2", target_bir_lowering=False, debug=False, num_devices=N_CORES
    )
    f32 = mybir.dt.float32
    f16 = mybir.dt.float16
    Copy = mybir.ActivationFunctionType.Copy
    add = mybir.AluOpType.add
    mult = mybir.AluOpType.mult

    x_d = nc.declare_dram_parameter("x", [B, CPC, H, W], f16, isOutput=False)
    xs_d = nc.declare_dram_parameter(
        "xs", [B, CPC, xs_rows, W], f16, isOutput=False
    )
    wm_d = nc.declare_dram_parameter(
        "wm", [CPC, KH * KW, CPC], f16, isOutput=False
    )
    wv_d = nc.declare_dram_parameter("wv", [CPC, 2, KH * KW], f32, isOutput=False)
    y_d = nc.declare_dram_parameter("y", [B, CPC, HO, WO], f16, isOutput=True)

    with tile.TileContext(nc) as tc:
        with (
            tc.tile_pool(name="wpool", bufs=1) as wpool,
            tc.tile_pool(name="xpool", bufs=2) as xpool,
            tc.tile_pool(name="xspool", bufs=2) as xspool,
            tc.tile_pool(name="oppool", bufs=2) as oppool,
            tc.tile_pool(name="odpool", bufs=2) as odpool,
            tc.tile_pool(name="ovpool", bufs=2) as ovpool,
            tc.tile_pool(name="tdpool", bufs=3) as tdpool,
            tc.tile_pool(name="tmpool", bufs=3) as tmpool,
            tc.tile_pool(name="psum", bufs=7, space="PSUM") as ppool,
            tc.tile_pool(name="scratch", bufs=1, space="PSUM") as spool,
        ):
            wt = wpool.tile([CPC, KH * KW, CPC], f16)
            # tap-0 weights land first so PE warmup starts ASAP
            nc.sync.dma_start(out=wt[:, 0:1, :], in_=wm_d[:, 0:1, :])
            wvt = wpool.tile([CPC, 2, KH * KW], f32)

            # The fused matmul (LDW+MM) supports only ONE semaphore wait;
            # sync matmuls absorb DMA waits so real matmuls only depend on
            # PE program order.
            scr = spool.tile([CPC, 512], f32)
            nc.tensor.matmul(
                scr[:, :2], lhsT=wt[:, 0, :], rhs=wt[:, 0, :2],
                start=True, stop=True,
            )
            # Dummy matmuls keep PE busy during the initial x DMA fill so
            # the HAM clock gate ramps to 2.4 GHz before real work arrives.
            for _ in range(cfg["WARMUP"]):
                nc.tensor.matmul(
                    scr[:, :128], lhsT=wt[:, 0, :], rhs=wt[:, 0, :],
                    start=True, stop=True,
                )

            taps = [
                (kh, kw, j)
                for kh in range(KH) for kw in range(KW) for j in range(2)
            ]
            n_supers = B // k

            def emit_loads(s):
                n0 = s * k
                xt = xpool.tile([CPC, k, H, W], f16)
                if s == 0:
                    # batch 0's first chunk rows land first; bulk weights
                    # stream right behind them
                    nc.sync.dma_start(
                        out=xt[:, 0, :11, :], in_=x_d[n0, :, :11, :]
                    )
                    nc.sync.dma_start(out=wt[:, 1:, :], in_=wm_d[:, 1:, :])
                    nc.sync.dma_start(
                        out=xt[:, 0, 11:HSPLIT, :], in_=x_d[n0, :, 11:HSPLIT, :]
                    )
                    nc.sync.dma_start(out=wvt[:], in_=wv_d[:])
                    nc.tensor.matmul(
                        scr[:, :2], lhsT=wt[:, 0, :], rhs=wt[:, 8, :2],
                        start=True, stop=True,
                    )
                    nc.sync.dma_start(
                        out=xt[:, 0, HSPLIT:, :], in_=x_d[n0, :, HSPLIT:, :]
                    )
                    rest = range(1, k)
                else:
                    rest = range(k)
                # xs first: it is small and gates the cross-term products
                xst = xspool.tile([CPC, k, xs_rows, W], f16)
                for b in range(k):
                    nc.sync.dma_start(out=xst[:, b], in_=xs_d[n0 + b])
                for b in rest:
                    nc.sync.dma_start(
                        out=xt[:, b, :HSPLIT, :], in_=x_d[n0 + b, :, :HSPLIT, :]
                    )
                    nc.sync.dma_start(
                        out=xt[:, b, HSPLIT:, :], in_=x_d[n0 + b, :, HSPLIT:, :]
                    )
                return xt, xst

            def emit_super(s, tiles, next_tiles_loader):
                n0 = s * k
                xt, xst = tiles
                # absorb x-DMA semaphores ahead of the real matmuls
                for b in range(k):
                    for row in (0, H - 1):
                        nc.tensor.matmul(
                            scr[:, :2], lhsT=wt[:, 0, :],
                            rhs=xt[:, b, row, :2], start=True, stop=True,
                        )

                # prefetch next super's inputs ahead of our y stores so the
                # loads never queue behind them on the DMA path
                nxt = next_tiles_loader() if next_tiles_loader else None

                ops = [
                    oppool.tile([CPC, r_pe, WO], f16, name=f"op{b}")
                    for b in range(k)
                ]
                ovd = odpool.tile([CPC, k, HO - r_pe, WO], f16)
                od = ovd[:, :, :r_dve, :]
                ov = ovd[:, :, r_dve:, :] if r_act else None
                n_ov2 = cfg["OV2"]
                ov2 = None
                if n_ov2:
                    ov2 = ovpool.tile([CPC, k, r_dve, WO], f16)

                def dve_slices(kh, kw, j):
                    if j == 0:
                        return xt[
                            :, :, vstart + kh : vstart + kh + r_dve,
                            kw : kw + WO,
                        ]
                    return xst[:, :, kh : kh + r_dve, kw : kw + WO]

                def emit_ov2_terms():
                    # out-of-chain accumulation of the last OV2 terms into
                    # ov2: ACT writes the first product directly, DVE
                    # produces + Pool adds the rest; one DVE merge at the
                    # end folds ov2 into od.
                    for m in range(n_ov2):
                        i = len(taps) - n_ov2 + m
                        kh, kw, j = taps[i]
                        t = kh * KW + kw
                        sc = wvt[:, j, t : t + 1]
                        dsl = dve_slices(kh, kw, j)
                        if m == 0:
                            nc.scalar.activation(ov2[:], dsl, Copy, scale=sc)
                        else:
                            t2 = tdpool.tile(
                                [CPC, k, r_dve, WO], f16, name="t2"
                            )
                            nc.vector.tensor_scalar(
                                out=t2[:], in0=dsl, scalar1=sc, scalar2=None,
                                op0=mult,
                            )
                            nc.gpsimd.tensor_tensor(
                                out=ov2[:], in0=ov2[:], in1=t2[:], op=add
                            )

                def emit_term(i):
                    kh, kw, j = taps[i]
                    t = kh * KW + kw
                    if j == 0:
                        dsl = xt[
                            :, :, vstart + kh : vstart + kh + r_dve,
                            kw : kw + WO,
                        ]
                        msl = xt[
                            :, :,
                            vstart + r_dve + kh : vstart + r_dve + kh + r_act,
                            kw : kw + WO,
                        ]
                    else:
                        dsl = xst[:, :, kh : kh + r_dve, kw : kw + WO]
                        msl = xst[
                            :, :, r_dve + kh : r_dve + kh + r_act, kw : kw + WO
                        ]
                    sc = wvt[:, j, t : t + 1]
                    if i == 0:
                        nc.vector.tensor_scalar(
                            out=od, in0=dsl, scalar1=sc, scalar2=None,
                            op0=mult,
                        )
                        if r_act:
                            nc.scalar.activation(ov, msl, Copy, scale=sc)
                    else:
                        td = tdpool.tile([CPC, k, r_dve, WO], f16)
                        nc.vector.tensor_scalar(
                            out=td[:], in0=dsl, scalar1=sc, scalar2=None,
                            op0=mult,
                        )
                        if i <= cfg["POOL_TT"]:
                            nc.gpsimd.tensor_tensor(
                                out=od, in0=od, in1=td[:], op=add
                            )
                        else:
                            nc.vector.tensor_tensor(
                                out=od, in0=od, in1=td[:], op=add
                            )
                        if r_act:
                            tm = tmpool.tile([CPC, k, r_act, WO], f16)
                            nc.scalar.activation(tm[:], msl, Copy, scale=sc)
                            nc.gpsimd.tensor_tensor(
                                out=ov, in0=ov, in1=tm[:], op=add
                            )

                if n_ov2:
                    emit_ov2_terms()
                ti = 0
                n_terms = len(taps) - n_ov2
                last_piece = OSPLIT
                for b in range(k):
                    op = ops[b]
                    for ci, (r0, nr) in enumerate(chunks):
                        pt = ppool.tile([CPC, nr, WO], f32)
                        t = 0
                        for kh in range(KH):
                            for kw in range(KW):
                                nc.tensor.matmul(
                                    pt[:],
                                    lhsT=wt[:, t, :],
                                    rhs=xt[
                                        :, b, r0 + kh : r0 + kh + nr,
                                        kw : kw + WO,
                                    ],
                                    start=(t == 0),
                                    stop=(t == KH * KW - 1),
                                )
                                t += 1
                        nc.scalar.activation(op[:, r0 : r0 + nr, :], pt[:], Copy)
                        if ti < n_terms:
                            emit_term(ti)
                            ti += 1
                        if r0 + nr == OSPLIT:
                            nc.sync.dma_start(
                                out=y_d[n0 + b, :, :OSPLIT, :],
                                in_=op[:, :OSPLIT, :],
                            )
                        elif ci == len(chunks) - 2:
                            nc.sync.dma_start(
                                out=y_d[n0 + b, :, OSPLIT : r0 + nr, :],
                                in_=op[:, OSPLIT : r0 + nr, :],
                            )
                            last_piece = r0 + nr
                while ti < n_terms:
                    emit_term(ti)
                    ti += 1
                if n_ov2:
                    nc.vector.tensor_tensor(
                        out=od, in0=od, in1=ov2[:], op=add
                    )

                # vector-region stores first: their producers run ahead of
                # the PE, so these must not queue behind the last yp piece
                for b in range(k):
                    nc.sync.dma_start(
                        out=y_d[n0 + b, :, vstart:, :], in_=ovd[:, b]
                    )
                for b in range(k):
                    nc.sync.dma_start(
                        out=y_d[n0 + b, :, last_piece:r_pe, :],
                        in_=ops[b][:, last_piece:, :],
                    )
                return nxt

            tiles = emit_loads(0)
            for s in range(n_supers):
                loader = (
                    (lambda s1=s + 1: emit_loads(s1))
                    if s + 1 < n_supers else None
                )
                tiles = emit_super(s, tiles, loader)
    nc.compile()
    return nc


def _get_nc():
    key = repr(sorted(CFG.items()))
    if key not in _NC_CACHE:
        _NC_CACHE[key] = _build_program(CFG)
    return _NC_CACHE[key]


def _make_wmats(w):
    """Per-core lhsT weight mats, shape (128, 9, 128): wm[ic, t, oc]."""
    oc = np.arange(CPC)
    mats = []
    for cid in range(N_CORES):
        ws = np.asarray(w[cid * CPC : (cid + 1) * CPC], dtype=np.float32)
        wm = np.zeros((CPC, KH * KW, CPC), dtype=np.float32)
        for icg in range(2):
            ic = (oc // 2) * 2 + icg
            wm[ic, :, oc] = ws[oc, icg].reshape(CPC, KH * KW)
        mats.append(wm.astype(np.float16))
    return mats


def _make_wvecs(w):
    """Per-core diag/cross scalar tables, shape (128, 2, 9) fp32."""
    p = np.arange(CPC)
    vecs = []
    for cid in range(N_CORES):
        ws = np.asarray(w[cid * CPC : (cid + 1) * CPC], dtype=np.float32)
        wv = np.empty((CPC, 2, KH * KW), dtype=np.float32)
        wv[:, 0, :] = ws[p, p % 2].reshape(CPC, KH * KW)
        wv[:, 1, :] = ws[p, 1 - p % 2].reshape(CPC, KH * KW)
        vecs.append(wv)
    return vecs


def _run(x, w, trace=False, **kwargs):
    nc = _get_nc()
    vstart = CFG["R_PE"]
    xs_rows = HO - vstart + KH - 1
    x = np.asarray(x)
    perm = np.arange(CPC) ^ 1
    wmats = _make_wmats(w)
    wvecs = _make_wvecs(w)
    in_maps = []
    for cid in range(N_CORES):
        xc = np.ascontiguousarray(
            x[:, cid * CPC : (cid + 1) * CPC], dtype=np.float16
        )
        xsc = np.ascontiguousarray(xc[:, perm, vstart : vstart + xs_rows, :])
        in_maps.append({"x": xc, "xs": xsc, "wm": wmats[cid], "wv": wvecs[cid]})
    res = run_bass_kernel_spmd(
        nc, in_maps, list(range(N_CORES)), trace=trace, **kwargs
    )
    y = np.concatenate(
        [res.results[i]["y"].astype(np.float32) for i in range(N_CORES)],
        axis=1,
    )
    return y, res


def kernel(x, w):
    y, _ = _run(x, w, trace=False)
    return y
